# revision 7
# baseline (speedup 1.0000x reference)
"""Fused bmm + residual kernel for Trainium2 (8 NeuronCores, data-parallel).

out[n,c,p] = x[n,c,p] + alpha * sum_q attn[n,p,q] * D[n,q,c]
  N=2048, C=512, H=W=7 (HW=49)

Sharding: batch N across 8 cores (256 each), no communication.

Per-core scheme (v3, fp32r big-N matmul):
 - x/out SBUF tiles [128, G, 196]: partition r holds channels {4r..4r+3}
   -> 784B-contiguous DMA runs at full 128 partitions.
 - D tiles [128, G/2, 512] f32r "gap" layout: partition b*64+q holds
   D[pair batch b, q, :]; gap rows stay zero.
 - attn pair transposed on PE ([49, 2x64-slot] -> [128, 49]): A^T(even)
   at rows 0:49, A^T(odd) at rows 64:113.
 - bd (stationary) [128, 2, 49] f32r: alpha*A^T(even) rows 0:49 block 0,
   alpha*A^T(odd) rows 64:113 block 1, zeros elsewhere.
 - ONE fp32r matmul per pair: out[98, 512] = bd[0:113].T @ D[0:113]
   (N=512 -> full-rate fp32r), i.e. y^T for both batches at once.
 - ACT copies y^T PSUM->SBUF; 4 PE transposes ([98, 128-strided] ->
   [128, 98]) restore channel-major with the 4r+j interleave.
 - residual add on DVE from PSUM, store via ACT-ring DMA.
"""
import sys

sys.path.insert(0, "/opt/trn_rl_repo")

import numpy as np

# ---- static problem config (hardcoded per harness contract) ----
N_TOT, C, HW = 2048, 512, 49
N_CORES = 8
NB = N_TOT // N_CORES        # 256 batches per core
G = 16                       # batches per group (one DMA round)
NPAIR = G // 2               # pairs per group
NGROUP = NB // G             # groups per core
NBD = 4                      # bd ring size
ND = 3                       # D-tile ring size

_cached = {}


def _build_bass():
    import concourse.bacc as bacc
    import concourse.mybir as mybir
    from concourse import tile

    f32 = mybir.dt.float32
    f32r = mybir.dt.float32r
    nc = bacc.Bacc(None, target_bir_lowering=False)

    x_d = nc.dram_tensor("x", [NB, C, HW], f32, kind="ExternalInput")
    a_d = nc.dram_tensor("attn", [NB, HW, HW], f32, kind="ExternalInput")
    d_d = nc.dram_tensor("d", [NB, HW, C], f32r, kind="ExternalInput")
    al_d = nc.dram_tensor("alphac", [128, 1], f32, kind="ExternalInput")
    id_d = nc.dram_tensor("ident", [2 * HW, 2 * HW], f32, kind="ExternalInput")
    o_d = nc.dram_tensor("out", [NB, C, HW], f32, kind="ExternalOutput")

    with tile.TileContext(nc) as tc:
        with (
            tc.tile_pool(name="const", bufs=1) as const,
            tc.tile_pool(name="bdp", bufs=NBD) as bdp,
            tc.tile_pool(name="dp", bufs=ND) as dp,
            tc.tile_pool(name="xp", bufs=3) as xp,
            tc.tile_pool(name="ap", bufs=3) as ap,
            tc.tile_pool(name="op", bufs=3) as op,
            tc.tile_pool(name="ytp", bufs=3) as ytp,
            tc.tile_pool(name="atp", bufs=2, space="PSUM") as atp,
            tc.tile_pool(name="ytps", bufs=2, space="PSUM") as ytps,
            tc.tile_pool(name="yp", bufs=3, space="PSUM") as yp,
        ):
            ident_sb = const.tile([2 * HW, 2 * HW], f32)
            nc.sync.dma_start(out=ident_sb, in_=id_d[:])
            alpha_sb = const.tile([128, 1], f32)
            nc.sync.dma_start(out=alpha_sb, in_=al_d[:])

            # bd ring: zeros except the two alpha*A^T blocks written per pair
            bd_tiles = []
            for i in range(NBD):
                t = bdp.tile([128, 2, HW], f32r, tag="bd")
                nc.vector.memset(t.bitcast(f32), 0.0)
                bd_tiles.append(t)

            # D ring: gap rows 49:64 / 113:128 kept zero (never read as
            # weights; the bd zero columns annihilate them anyway)
            d_tiles = []
            for i in range(ND):
                t = dp.tile([128, NPAIR, C], f32r, tag="d")
                nc.vector.memset(t[32:64, :, :].bitcast(f32), 0.0)
                nc.vector.memset(t[96:128, :, :].bitcast(f32), 0.0)
                d_tiles.append(t)

            for g in range(NGROUP):
                b0 = g * G
                xs = x_d[b0:b0 + G]      # [G, C, HW]
                os_ = o_d[b0:b0 + G]
                ds = d_d[b0:b0 + G]      # [G, HW, C]
                as_ = a_d[b0:b0 + G]     # [G, HW, HW]

                x_t = xp.tile([128, G, 4 * HW], f32, tag="x")
                nc.sync.dma_start(
                    out=x_t, in_=xs.rearrange("n (r j) p -> r n (j p)", j=4)
                )
                d_t = d_tiles[g % ND]
                d_v = d_t.rearrange("(b s) i c -> b s i c", b=2)
                dsr = ds.rearrange("(i b) q c -> b q i c", b=2)
                nc.sync.dma_start(out=d_v[0, 0:HW, :, :], in_=dsr[0])
                nc.sync.dma_start(out=d_v[1, 0:HW, :, :], in_=dsr[1])
                # attn in 64-wide slots so the pair transpose lands the odd
                # batch at PSUM rows 64:113
                a_t = ap.tile([HW, G, 64], f32, tag="a")
                nc.sync.dma_start(
                    out=a_t[:, :, 0:HW], in_=as_.rearrange("n p q -> p n q")
                )

                o_t = op.tile([128, G, 4 * HW], f32, tag="o")

                x4 = x_t.rearrange("r n (j p) -> r n j p", j=4)
                o4 = o_t.rearrange("r n (j p) -> r n j p", j=4)
                a2 = a_t.rearrange("p n q -> p (n q)")

                for i in range(NPAIR):
                    at_ps = atp.tile([128, HW], f32, tag="at")
                    # [49, 128] -> [128, 49]: rows b*64+q = A^T pair
                    nc.tensor.transpose(
                        at_ps,
                        a2[:, 2 * i * 64:(2 * i + 2) * 64],
                        ident_sb[0:HW, 0:HW],
                    )
                    bd = bd_tiles[i % NBD]
                    nc.vector.tensor_scalar_mul(
                        out=bd[0:HW, 0, :],
                        in0=at_ps[0:HW, :],
                        scalar1=alpha_sb[0:HW, :],
                    )
                    nc.vector.tensor_scalar_mul(
                        out=bd[64:64 + HW, 1, :],
                        in0=at_ps[64:64 + HW, :],
                        scalar1=alpha_sb[64:64 + HW, :],
                    )

                    # one fp32r matmul: y^T[b*49+p, c] for both batches
                    yt_ps = ytps.tile([2 * HW, C], f32, tag="yt")
                    bd2 = bd.rearrange("k b p -> k (b p)")
                    nc.tensor.matmul(
                        out=yt_ps,
                        lhsT=bd2[0:64 + HW, :],
                        rhs=d_t[0:64 + HW, i, :],
                        start=True,
                        stop=True,
                    )
                    yt_sb = ytp.tile([2 * HW, C], f32, tag="yts")
                    nc.scalar.copy(out=yt_sb, in_=yt_ps)

                    # 4 transposes: [98, 128 (c=4m+j)] -> [128, 98 (b p)]
                    y_ps = yp.tile([128, 4, 2 * HW], f32, tag="y")
                    ytv = yt_sb.rearrange("k (m four) -> k four m", four=4)
                    for j in range(4):
                        nc.tensor.transpose(
                            y_ps[:, j, :], ytv[:, j, :], ident_sb
                        )
                    # y_ps free layout: (j, b, p); regroup to (b, j, p)
                    y4 = y_ps.rearrange("r j (b p) -> r b j p", b=2)
                    nc.vector.tensor_add(
                        out=o4[:, 2 * i:2 * i + 2, :, :],
                        in0=y4,
                        in1=x4[:, 2 * i:2 * i + 2, :, :],
                    )

                nc.scalar.dma_start(
                    out=os_.rearrange("n (r j) p -> r n (j p)", j=4), in_=o_t
                )

    nc.finalize()
    return nc


def _get_nc():
    if "nc" not in _cached:
        _cached["nc"] = _build_bass()
    return _cached["nc"]


def _in_maps(x, attn, D, alpha):
    x_s = np.ascontiguousarray(x, dtype=np.float32).reshape(N_CORES, NB, C, HW)
    a_s = np.ascontiguousarray(attn, dtype=np.float32).reshape(N_CORES, NB, HW, HW)
    d_s = np.ascontiguousarray(D, dtype=np.float32).reshape(N_CORES, NB, HW, C)
    al = np.full((128, 1), np.float32(np.asarray(alpha).reshape(-1)[0]), np.float32)
    ident = np.eye(2 * HW, dtype=np.float32)
    return [
        {"x": x_s[c], "attn": a_s[c], "d": d_s[c], "alphac": al, "ident": ident}
        for c in range(N_CORES)
    ]


def kernel(x: np.ndarray, attn: np.ndarray, D: np.ndarray, alpha: np.ndarray) -> np.ndarray:
    from concourse import bass_utils

    nc = _get_nc()
    res = bass_utils.run_bass_kernel_spmd(
        nc, _in_maps(x, attn, D, alpha), core_ids=list(range(N_CORES))
    )
    out = np.stack([res.results[c]["out"] for c in range(N_CORES)])
    return out.reshape(N_TOT, C, 7, 7).astype(np.float32, copy=False)
